# revision 34
# baseline (speedup 1.0000x reference)
"""Sequence-parallel Trainium2 attention-head kernel (softmax over queries).

Shard the QUERY dim across the 8 cores (slab = 1024 queries each); every
core computes scores for ALL 8192 keys x its own query slab with j (keys)
on partitions, so the softmax-over-queries stats are per-partition-row:

  out[i,:] = sum_j exp(s_ij - M_j)/D_j * v[j,:],  M_j/D_j global over i.

Per core:
  1. Project its k/v slab (j on partitions) -> AllGather #1 (384 KB/core)
     so every core holds full projected KS=[kh;kl] and v.  Project its q
     slab -> Q1=[qh;ql], Q2=[ql;qh] (weights host-duplicated into both
     column halves so the fp32 projection lands on all 128 partitions in
     one pass; hi/lo split is then partition-aligned DVE).
  2. Per j-tile t (128 keys x 1024 local queries): 2 stacked matmuls
     (KS_t^T Q1 + KS_t^T Q2 = all 4 hi/lo cross terms), DVE negated
     row-max -> bias, ACT exp (psum->bf16) with accum_out -> local sums.
     Bias is the LOCAL max over this core's slab, so the per-row rescale
     exp(b_local - M)/D folds entirely into v later.
  3. AllGather #2 of the per-(j,core) stats [negmax|sum] (64 KB/core);
     every core reduces them to global M_j, D_j, rescales v rows by
     f_j = exp(b_local_j - M_j)/D_j, then accumulates the 64 attn
     matmuls v'_t^T e_t into psum [64 v, 1024 i] and writes its slab.
"""

import numpy as np

C = 8
QK = 64
VD = 64


def build_nc2(seq=8192, d=1024, reps=1, warmup=True, dbg=False, noscale=False, attn_lite=False, scores_lite=False, no_cc=False, exp_lite=False, max_lite=False):
    import concourse.bacc as bacc
    import concourse.mybir as mybir

    f32 = mybir.dt.float32
    bf16 = mybir.dt.bfloat16
    AX = mybir.AxisListType.X
    ALU = mybir.AluOpType
    ACTF = mybir.ActivationFunctionType

    NDT = d // 128            # 8 d-tiles
    SLAB = seq // C           # 1024 queries per core
    NT = seq // 128           # 64 global j-tiles
    NST = SLAB // 128         # 8 v-proj subtiles per slab
    SS = max(SLAB, 512)       # psum slot stride (bank-aligned)
    NDMA = 3 * NDT            # arena chunk DMAs per iteration (k, v, q)
    INF = 3.0e38

    nc = bacc.Bacc(target_bir_lowering=False, debug=False, num_devices=C)

    def din(name, w, dt=bf16):
        return nc.declare_dram_parameter(name, [128, w], dt, isOutput=False)

    qin_d = din("qin", NDT * 2 * SLAB)     # col = dd*2048 + h*1024 + i
    kin_d = din("kin", NDT * 2 * SLAB)
    vin_d = din("vin", NDT * SLAB)         # col = dd*1024 + j
    wq2h_d, wq2l_d = din("wq2h", NDT * 128), din("wq2l", NDT * 128)
    wk2h_d, wk2l_d = din("wk2h", NDT * 128), din("wk2l", NDT * 128)
    wv_d = din("wv", NDT * VD)
    out_d = nc.declare_dram_parameter("out", [VD, SLAB], f32, isOutput=True)
    if dbg:
        dKS_d = nc.declare_dram_parameter("dKS", [128, seq], bf16, isOutput=True)
        dQ1_d = nc.declare_dram_parameter("dQ1", [128, SLAB], bf16, isOutput=True)
        dQ2_d = nc.declare_dram_parameter("dQ2", [128, SLAB], bf16, isOutput=True)
        dst_d = nc.declare_dram_parameter("dst", [128, 2 * NT], f32, isOutput=True)
        dsa_d = nc.declare_dram_parameter("dsa", [128, C * 2 * NT], f32, isOutput=True)
        df_d = nc.declare_dram_parameter("df", [128, NT], f32, isOutput=True)
        dvl_d = nc.declare_dram_parameter("dvl", [128, SLAB // 2], bf16, isOutput=True)
        dva_d = nc.declare_dram_parameter("dva", [128, NT * VD], bf16, isOutput=True)
        de_d = nc.declare_dram_parameter("de", [128, 4 * SLAB], bf16, isOutput=True)

    # collective bounce buffers (DRAM; collectives can't touch I/O tensors)
    NKG = min(4, SLAB // 128) # KS gather chunks (pipelined collectives)
    KGW = SLAB // NKG         # j-columns per chunk
    cc1_ins = [
        nc.dram_tensor(f"cc1i_{g}", [128, KGW], bf16) for g in range(NKG)
    ]
    ccv_in = nc.dram_tensor("ccv_in", [128, SLAB // 2], bf16)
    cc1_outs = [
        nc.dram_tensor(f"cc1o_{g}", [C * 128, KGW], bf16, addr_space="Shared")
        for g in range(NKG)
    ]
    ccv_out = nc.dram_tensor(
        "ccv_out", [C * 128, SLAB // 2], bf16, addr_space="Shared"
    )
    cc2_ins = [nc.dram_tensor(f"cc2i_{h}", [128, NT], f32) for h in range(2)]
    cc2_outs2 = [
        nc.dram_tensor(f"cc2o_{h}", [C * 128, NT], f32, addr_space="Shared")
        for h in range(2)
    ]
    HW2 = NT // 2

    from contextlib import ExitStack

    with ExitStack() as ctx:
        block = ctx.enter_context(nc.Block())
        sem = lambda n: ctx.enter_context(nc.semaphore(n))
        sb = lambda n, shape, dt: ctx.enter_context(nc.sbuf_tensor(n, shape, dt))
        ps = lambda n, shape: ctx.enter_context(nc.psum_tensor(n, shape, f32))

        s_w = sem("s_w")        # weight DMAs: 80 once
        s_ka = [sem(f"s_ka{i}") for i in range(NDT)]   # per-chunk k DMA
        s_va = [sem(f"s_va{i}") for i in range(NST)]   # per-chunk v DMA
        s_qa = [sem(f"s_qa{i}") for i in range(NDT)]   # per-chunk q DMA
        s_kp = sem("s_kp")      # 8/iter (k proj per dd)
        s_vp = sem("s_vp")      # 8/iter
        s_qp = sem("s_qp")      # 8/iter
        s_ks = sem("s_ks")      # 1/iter k split done
        s_vsp = sem("s_vsp")    # 1/iter vloc copy done
        s_qs = sem("s_qs")      # 1/iter q split done
        s_gb = sem("s_gb")      # 32/iter bounce-in DMAs
        s_cc1 = sem("s_cc1")    # 1/iter
        s_ccd = sem("s_ccd")    # 256/iter gather-back DMAs
        s_sc = sem("s_sc")      # 64/iter score tiles
        s_mx = sem("s_mx")      # 64/iter
        s_ex = sem("s_ex")      # 64/iter
        s_g2 = sem("s_g2")      # 16/iter stats bounce DMA
        s_cc2 = sem("s_cc2")    # 1/iter
        s_std = sem("s_std")    # 128/iter stats gather-back
        s_sm = sem("s_sm")      # 1/iter DVE->ACT stats handoff
        s_sa = sem("s_sa")      # 2/iter ACT->DVE stats handoff
        s_f = sem("s_f")        # 2/iter DVE f ready (per half)
        s_vsc = sem("s_vsc")    # 64/iter v rescales
        s_at = sem("s_at")      # 1/iter attn done
        s_oc = sem("s_oc")      # 1/iter out copy done
        s_out = sem("s_out")    # 16/iter out DMA
        s_ch = sem("s_ch")      # DVE same-engine RAW chain

        arena_k = sb("arena_k", [128, 2 * 2 * SLAB], bf16)  # dd%2 ping-pong
        arena_v = sb("arena_v", [128, 2 * NDT * 128], bf16)
        arena_q = sb("arena_q", [128, 2 * 2 * SLAB], bf16)
        wq2h = sb("wq2h_s", [128, NDT * 128], bf16)
        wq2l = sb("wq2l_s", [128, NDT * 128], bf16)
        wk2h = sb("wk2h_s", [128, NDT * 128], bf16)
        wk2l = sb("wk2l_s", [128, NDT * 128], bf16)
        wv = sb("wv_s", [128, NDT * VD], bf16)
        Q1 = sb("Q1", [128, SLAB], bf16)
        Q2 = sb("Q2", [128, SLAB], bf16)
        KSloc = sb("KSloc", [128, SLAB], bf16)
        ktmp = sb("ktmp", [128, SLAB], bf16)
        vloc = sb("vloc", [128, SLAB // 2], bf16)
        KS = sb("KS", [128, seq], bf16)
        v_all = sb("v_all", [128, NT * VD], bf16)
        e_sb = sb("e_sb", [128, NT * SLAB], bf16)           # 128 KB/part
        stats_loc = sb("stats_loc", [128, 2 * NT], f32)     # [negmax | sum]
        stats_all = sb("stats_all", [128, C * 2 * NT], f32)
        NM = sb("NM", [128, NT], f32)
        darg = sb("darg", [128, C * NT], f32)
        w8 = sb("w8", [128, C * NT], f32)
        wD = sb("wD", [128, C * NT], f32)
        Dt = sb("Dt", [128, NT], f32)
        Rt = sb("Rt", [128, NT], f32)
        wlarg = sb("wlarg", [128, NT], f32)
        wl = sb("wl", [128, NT], f32)
        f_sb = sb("f_sb", [128, NT], f32)

        out_sb = sb("out_sb", [VD, SLAB], f32)
        ps_s = ps("ps_s", [128, 3 * SS])     # proj + score slots (6 banks)
        ps_o = ps("ps_o", [VD, SLAB])        # 2 banks
        ps_v = ps_s[:, 2 * SS : 2 * SS + SLAB // 2]   # v proj in slot 2

        TORD = [
            c * (SLAB // 128) + gi * (KGW // 128) + u
            for gi in range(NKG)
            for c in range(C)
            for u in range(KGW // 128)
        ]

        # ---------------- SYNC: k/v input + output DMAs ----------------
        # per-chunk s_in levels must be ordered sync points, so each arena
        # DMA is serialized (waited) before the next is issued.
        @block.sync
        def _(s):
          for it in range(reps):
            for dd in range(NDT):          # k chunks
                g = it * NDT + dd
                if g >= 2:
                    s.wait_ge(s_kp, g - 1)
                s.dma_start(
                    out=arena_k[:, (dd % 2) * 2 * SLAB : (dd % 2 + 1) * 2 * SLAB],
                    in_=kin_d[:, dd * 2 * SLAB : (dd + 1) * 2 * SLAB],
                ).then_inc(s_ka[dd], 16)
            for st in range(NST):          # v chunks (one j-subtile, all dd)
                g = it * NST + st
                if g >= 2:
                    s.wait_ge(s_vp, g - 1)
                s.dma_start(
                    out=arena_v[:, (st % 2) * NDT * 128 : (st % 2 + 1) * NDT * 128],
                    in_=vin_d[:, st * NDT * 128 : (st + 1) * NDT * 128],
                ).then_inc(s_va[st], 16)
            s.wait_ge(s_oc, it + 1)
            s.dma_start(out=out_d[:, :], in_=out_sb[:, :]).then_inc(s_out, 16)
            s.wait_ge(s_out, it * 16 + 16)
            if dbg and it == reps - 1:
                n_d = 0
                for dst, srcb in ((dKS_d, KS), (dQ1_d, Q1), (dQ2_d, Q2),
                                  (dst_d, stats_loc), (dsa_d, stats_all),
                                  (df_d, f_sb), (dva_d, v_all),
                                  (dvl_d, vloc)):
                    s.dma_start(out=dst[:, :], in_=srcb[:, :]).then_inc(s_out, 16)
                    n_d += 1
                s.dma_start(
                    out=de_d[:, :], in_=e_sb[:, 0 : 4 * SLAB]
                ).then_inc(s_out, 16)
                n_d += 1
                s.wait_ge(s_out, it * 16 + 16 + n_d * 16)

        # ---------------- TENSOR (PE) ----------------
        @block.tensor
        def _(t):
          for it in range(reps):
            for w_ in range(40 if (warmup and it == 0) else 0):
                t.matmul(
                    ps_s[0:64, 0:512], Q1[:, 0:64], Q1[:, 0 : min(SLAB, 512)],
                    start=(w_ == 0), stop=False,
                )
            if it == 0:
                t.wait_ge(s_w, 80)
            # ---- k proj -> ps_s[:, 0:SLAB] (both halves via dup weights)
            if it > 0:
                t.wait_ge(s_ex, it * NT)       # all prev-rep slots free
            for dd in range(NDT):
                t.wait_ge(s_ka[dd], (it + 1) * 16)
                kb = (dd % 2) * 2 * SLAB
                uh = arena_k[:, kb : kb + SLAB]
                ul = arena_k[:, kb + SLAB : kb + 2 * SLAB]
                for ti, (W, X) in enumerate(((wk2h, uh), (wk2h, ul), (wk2l, uh))):
                    for hb in range(0, SLAB, 512):
                        he = min(hb + 512, SLAB)
                        mm = t.matmul(
                            ps_s[:, hb:he],
                            W[:, dd * 128 : (dd + 1) * 128], X[:, hb:he],
                            start=(dd == 0 and ti == 0),
                            stop=(dd == NDT - 1 and ti == 2),
                            skip_group_check=True,
                        )
                mm.then_inc(s_kp, 1)
            # ---- v proj -> ps_v (st-outer: one sequential group per
            # subtile; vin is re-tiled so chunk st holds all dd for its j's)
            if it > 0:
                t.wait_ge(s_vsp, it)           # ps_v free
            for st in range(NST):
                t.wait_ge(s_va[st], (it + 1) * 16)
                vb = (st % 2) * NDT * 128
                for dd in range(NDT):
                    mm = t.matmul(
                        ps_v[:, st * VD : (st + 1) * VD],
                        arena_v[:, vb + dd * 128 : vb + (dd + 1) * 128],
                        wv[:, dd * VD : (dd + 1) * VD],
                        start=(dd == 0), stop=(dd == NDT - 1),
                    )
                mm.then_inc(s_vp, 1)
            # ---- q proj -> ps_s[:, SLAB:2*SLAB]
            if it > 0:
                t.wait_ge(s_ex, it * NT)       # slot1 free (prev tile 63)
            for dd in range(NDT):
                t.wait_ge(s_qa[dd], (it + 1) * 16)
                qb = (dd % 2) * 2 * SLAB
                uh = arena_q[:, qb : qb + SLAB]
                ul = arena_q[:, qb + SLAB : qb + 2 * SLAB]
                for ti, (W, X) in enumerate(((wq2h, uh), (wq2h, ul), (wq2l, uh))):
                    for hb in range(0, SLAB, 512):
                        he = min(hb + 512, SLAB)
                        mm = t.matmul(
                            ps_s[:, SS + hb : SS + he],
                            W[:, dd * 128 : (dd + 1) * 128], X[:, hb:he],
                            start=(dd == 0 and ti == 0),
                            stop=(dd == NDT - 1 and ti == 2),
                            skip_group_check=True,
                        )
                mm.then_inc(s_qp, 1)
            # ---- scores: 64 j-tiles, ordered by gather chunk so tiles of
            # chunk gi run as soon as collective gi has landed
            t.wait_ge(s_qs, it + 1)
            for pos in range(NT):
                tt = TORD[pos]
                gi = pos // (NT // NKG)
                if pos % (NT // NKG) == 0:
                    t.wait_ge(s_ccd, it * (NKG + 1) * 16 + (gi + 1) * 16)
                g = it * NT + pos
                if g >= 3:
                    t.wait_ge(s_ex, g - 2)     # slot free (exp 3 positions ago)
                so = (pos % 3) * SS
                kt = KS[:, tt * 128 : (tt + 1) * 128]
                for hb in range(0, SLAB, 512):
                    he = min(hb + 512, SLAB)
                    if not scores_lite:
                        t.matmul(
                            ps_s[:, so + hb : so + he], kt, Q1[:, hb:he],
                            start=True, stop=False, skip_group_check=True,
                        )
                    mm = t.matmul(
                        ps_s[:, so + hb : so + he], kt, Q2[:, hb:he],
                        start=scores_lite, stop=True, skip_group_check=True,
                    )
                mm.then_inc(s_sc, 1)
                # interleaved attn for the first half's tiles once their
                # stats have come back (pos 58..63 -> attn 0..5)
                if NT >= 32 and pos >= NT - 6:
                    p = pos - (NT - 6)
                    at = TORD[p]
                    if p == 0:
                        t.wait_ge(s_oc, it)    # ps_o free
                    t.wait_ge(s_vsc, it * NT + p + 1)
                    for hb in range(0, SLAB, 512):
                        he = min(hb + 512, SLAB)
                        t.matmul(
                            ps_o[:, hb:he],
                            v_all[:, at * VD : (at + 1) * VD],
                            e_sb[:, at * SLAB + hb : at * SLAB + he],
                            start=(p == 0), stop=False,
                            skip_group_check=True,
                        )
            # ---- attn tail: remaining tiles
            for p in range(6 if NT >= 32 else 0, NT):
                at = TORD[p]
                t.wait_ge(s_vsc, it * NT + p + 1)
                if p == 0:
                    t.wait_ge(s_oc, it)        # ps_o free
                for hb in range(0, SLAB, 512):
                    he = min(hb + 512, SLAB)
                    mm = t.matmul(
                        ps_o[:, hb:he],
                        v_all[:, at * VD : (at + 1) * VD],
                        e_sb[:, at * SLAB + hb : at * SLAB + he],
                        start=(p == 0), stop=(p == NT - 1),
                        skip_group_check=True,
                    )
            mm.then_inc(s_at, 1)

        # ---------------- VECTOR (DVE) ----------------
        ch_n = [0]

        @block.vector
        def _(v):
          def step(inst):
              inst.then_inc(s_ch, 1)
              ch_n[0] += 1

          def cw(v):
              v.wait_ge(s_ch, ch_n[0])

          for it in range(reps):
            # k split -> KSloc = [kh; kl]
            v.wait_ge(s_kp, it * NDT + NDT)
            if it > 0:
                v.wait_ge(s_gb, it * (NKG + 1) * 16)   # KSloc/vloc consumed
            step(v.tensor_copy(KSloc[0:64, :], ps_s[0:64, 0:SLAB]))
            step(v.tensor_copy(ktmp[64:128, :], ps_s[64:128, 0:SLAB]))
            cw(v)
            v.tensor_tensor(
                KSloc[64:128, :], ps_s[64:128, 0:SLAB], ktmp[64:128, :],
                op=ALU.subtract,
            ).then_inc(s_ks, 1)
            # v copy
            v.wait_ge(s_vp, it * NST + NST)
            v.tensor_copy(vloc[:, :], ps_v[:, :]).then_inc(s_vsp, 1)
            # q split -> Q1=[qh;ql], Q2=[ql;qh]
            v.wait_ge(s_qp, it * NDT + NDT)
            qsl = ps_s[:, SS : SS + SLAB]
            step(v.tensor_copy(Q1[0:64, :], qsl[0:64, :]))
            step(v.tensor_copy(Q2[64:128, :], qsl[64:128, :]))
            cw(v)
            v.tensor_tensor(
                Q1[64:128, :], qsl[64:128, :], Q2[64:128, :], op=ALU.subtract
            )
            v.tensor_tensor(
                Q2[0:64, :], qsl[0:64, :], Q1[0:64, :], op=ALU.subtract
            ).then_inc(s_qs, 1)
            # per-tile negated row-max: one fused pass over both psum
            # halves: scr = -max(lo, hi) elementwise, nb = min(scr)
            def part1(h):
                v.wait_ge(s_std, it * 64 + h * 32 + 32)
                nb_v = stats_all[:, :].rearrange(
                    "p (c t) -> p t c", c=C, t=2 * NT
                )
                hs = slice(h * HW2, (h + 1) * HW2)
                step(v.tensor_reduce(
                    NM[:, hs], nb_v[:, h * HW2 : (h + 1) * HW2, :],
                    axis=AX, op=ALU.min,
                ))
                cw(v)
                for c in range(C):
                    v.tensor_tensor(
                        darg[:, c * NT + h * HW2 : c * NT + (h + 1) * HW2],
                        NM[:, hs],
                        stats_all[:, c * 2 * NT + h * HW2 : c * 2 * NT + (h + 1) * HW2],
                        op=ALU.subtract,
                    )
                v.tensor_tensor(
                    wlarg[:, hs], NM[:, hs], stats_loc[:, hs], op=ALU.subtract
                ).then_inc(s_sm, 1)

            def part2(h):
                v.wait_ge(s_sa, it * 2 + h + 1)
                hs = slice(h * HW2, (h + 1) * HW2)
                dl_v = stats_all[:, :].rearrange(
                    "p (c t) -> p c t", c=C, t=2 * NT
                )
                step(v.tensor_tensor(
                    wD[:, :].rearrange("p (c t) -> p c t", c=C, t=NT)[
                        :, :, h * HW2 : (h + 1) * HW2
                    ],
                    w8[:, :].rearrange("p (c t) -> p c t", c=C, t=NT)[
                        :, :, h * HW2 : (h + 1) * HW2
                    ],
                    dl_v[:, :, NT + h * HW2 : NT + (h + 1) * HW2],
                    op=ALU.mult,
                ))
                cw(v)
                step(v.tensor_reduce(
                    Dt[:, hs],
                    wD[:, :].rearrange("p (c t) -> p t c", c=C, t=NT)[
                        :, h * HW2 : (h + 1) * HW2, :
                    ],
                    axis=AX, op=ALU.add,
                ))
                cw(v)
                step(v.reciprocal(Rt[:, hs], Dt[:, hs]))
                cw(v)
                step(v.tensor_tensor(
                    f_sb[:, hs], wl[:, hs], Rt[:, hs], op=ALU.mult
                ))
                cw(v)
                v.nop().then_inc(s_f, 1)

            for pos in range(NT):
                v.wait_ge(s_sc, it * NT + pos + 1)
                if pos == 0 and it > 0:
                    v.wait_ge(s_g2, it * 64)   # stats_loc consumed
                    v.wait_ge(s_at, it)        # e_sb junk-write target free
                so = (pos % 3) * SS
                v.tensor_reduce(
                    stats_loc[:, pos : pos + 1],
                    ps_s[:, so : so + (SLAB // 2 if max_lite else SLAB)],
                    axis=AX, op=ALU.max, negate=True,
                ).then_inc(s_mx, 1)
                if NT >= 64 and pos == 52:
                    part1(0)
                if NT >= 64 and pos == 56:
                    part2(0)
            if NT < 64:
                part1(0)
                part2(0)
            part1(1)
            part2(1)

        # ---------------- SCALAR (ACT) ----------------
        @block.scalar
        def _(sc):
          for it in range(reps):
            # q arena stream on the ACT queue (serialized per-chunk levels)
            for dd in range(NDT):
                g = it * NDT + dd
                if g >= 2:
                    sc.wait_ge(s_qp, g - 1)
                sc.dma_start(
                    out=arena_q[:, (dd % 2) * 2 * SLAB : (dd % 2 + 1) * 2 * SLAB],
                    in_=qin_d[:, dd * 2 * SLAB : (dd + 1) * 2 * SLAB],
                ).then_inc(s_qa[dd], 16)
            def half_exps(h):
                sc.wait_ge(s_sm, it * 2 + h + 1)
                hs = slice(h * HW2, (h + 1) * HW2)
                sc.activation(
                    w8[:, :].rearrange("p (c t) -> p c t", c=C, t=NT)[
                        :, :, h * HW2 : (h + 1) * HW2
                    ],
                    darg[:, :].rearrange("p (c t) -> p c t", c=C, t=NT)[
                        :, :, h * HW2 : (h + 1) * HW2
                    ],
                    ACTF.Exp,
                )
                sc.activation(wl[:, hs], wlarg[:, hs], ACTF.Exp).then_inc(
                    s_sa, 1
                )

            for pos in range(NT):
                tt = TORD[pos]
                sc.wait_ge(s_mx, it * NT + pos + 1)
                if pos == 0 and it > 0:
                    sc.wait_ge(s_at, it)       # e_sb consumed by attn
                ew = SLAB // 2 if exp_lite else SLAB
                sc.activation(
                    e_sb[:, tt * SLAB : tt * SLAB + ew],
                    ps_s[:, (pos % 3) * SS : (pos % 3) * SS + ew],
                    ACTF.Exp,
                    bias=stats_loc[:, pos : pos + 1], scale=1.0,
                    accum_out=stats_loc[:, NT + pos : NT + pos + 1],
                ).then_inc(s_ex, 1)
                if NT >= 64 and pos == 54:
                    half_exps(0)
            if NT < 64:
                half_exps(0)
            half_exps(1)
            sc.wait_ge(s_at, it + 1)
            if it > 0:
                sc.wait_ge(s_out, it * 16)     # out_sb consumed
            sc.activation(out_sb[:, :], ps_o[:, :], ACTF.Copy).then_inc(s_oc, 1)

        # ---------------- GPSIMD: collectives ----------------
        @block.gpsimd
        def _(g):
          for it in range(reps):
            if it == 0:
                for wsb, wdr in ((wq2h, wq2h_d), (wq2l, wq2l_d),
                                 (wk2h, wk2h_d), (wk2l, wk2l_d), (wv, wv_d)):
                    g.dma_start(out=wsb[:, :], in_=wdr[:, :]).then_inc(s_w, 16)
            g.wait_ge(s_ks, it + 1)
            g.wait_ge(s_vsp, it + 1)
            if it > 0:
                g.wait_ge(s_cc1, it * (NKG + 1))   # cc1_in consumed
            for gi in range(NKG):
                g.dma_start(
                    out=cc1_ins[gi][:, :],
                    in_=KSloc[:, gi * KGW : (gi + 1) * KGW],
                ).then_inc(s_gb, 16)
            g.dma_start(out=ccv_in[:, :], in_=vloc[:, :]).then_inc(s_gb, 16)
            g.wait_ge(s_gb, it * (NKG + 1) * 16 + (NKG + 1) * 16)
            for gi in range(NKG):
                if no_cc:
                    g.nop().then_inc(s_cc1, 1)
                else:
                    g.collective_compute(
                        "AllGather", mybir.AluOpType.bypass,
                        replica_groups=[list(range(C))],
                        ins=[cc1_ins[gi][:, :].opt()],
                        outs=[cc1_outs[gi][:, :].opt()],
                    ).then_inc(s_cc1, 1)
                g.wait_ge(s_cc1, it * (NKG + 1) + gi + 1)
                g.wait_ge(s_ccd, it * (NKG + 1) * 16 + gi * 16)
                g.dma_start(
                    out=KS[:, :].rearrange("p (c s) -> p c s", c=C)[
                        :, :, gi * KGW : (gi + 1) * KGW
                    ],
                    in_=cc1_outs[gi][:, :].rearrange("(c p) w -> p c w", c=C),
                ).then_inc(s_ccd, 16)
            if no_cc:
                g.nop().then_inc(s_cc1, 1)
            else:
                g.collective_compute(
                    "AllGather", mybir.AluOpType.bypass,
                    replica_groups=[list(range(C))],
                    ins=[ccv_in[:, :].opt()],
                    outs=[ccv_out[:, :].opt()],
                ).then_inc(s_cc1, 1)
            g.wait_ge(s_cc1, it * (NKG + 1) + NKG + 1)
            g.wait_ge(s_ccd, it * (NKG + 1) * 16 + NKG * 16)
            g.dma_start(
                out=v_all[:, :].rearrange("p (c s) -> p c s", c=C),
                in_=ccv_out[:, :].rearrange("(c p) w -> p c w", c=C),
            ).then_inc(s_ccd, 16)
            # stats gather + v rescale, per half (overlaps the score phase)
            sa_v = stats_all[:, :].rearrange("p (c s) -> p c s", c=C)
            for h in range(2):
                g.wait_ge(s_ex, it * NT + (h + 1) * HW2)
                if it > 0:
                    g.wait_ge(s_cc2, it * 2)   # cc2_ins free
                g.dma_start(
                    out=cc2_ins[h][:, 0:HW2],
                    in_=stats_loc[:, h * HW2 : (h + 1) * HW2],
                ).then_inc(s_g2, 16)
                g.dma_start(
                    out=cc2_ins[h][:, HW2 : 2 * HW2],
                    in_=stats_loc[:, NT + h * HW2 : NT + (h + 1) * HW2],
                ).then_inc(s_g2, 16)
                g.wait_ge(s_g2, it * 64 + h * 32 + 32)
                if no_cc:
                    g.nop().then_inc(s_cc2, 1)
                else:
                    g.collective_compute(
                        "AllGather", mybir.AluOpType.bypass,
                        replica_groups=[list(range(C))],
                        ins=[cc2_ins[h][:, :].opt()],
                        outs=[cc2_outs2[h][:, :].opt()],
                    ).then_inc(s_cc2, 1)
                g.wait_ge(s_cc2, it * 2 + h + 1)
                g.wait_ge(s_std, it * 64 + h * 32)
                co_v = cc2_outs2[h][:, :].rearrange("(c p) w -> p c w", c=C)
                g.dma_start(
                    out=sa_v[:, :, h * HW2 : (h + 1) * HW2],
                    in_=co_v[:, :, 0:HW2],
                ).then_inc(s_std, 16)
                g.dma_start(
                    out=sa_v[:, :, NT + h * HW2 : NT + (h + 1) * HW2],
                    in_=co_v[:, :, HW2 : 2 * HW2],
                ).then_inc(s_std, 16)
                # v rescale for this half's tiles
                g.wait_ge(s_f, it * 2 + h + 1)
                if h == 0:
                    g.wait_ge(s_ccd, (it + 1) * (NKG + 1) * 16)  # v_all landed
                for p in range(h * HW2, (h + 1) * HW2):
                    at = TORD[p]
                    g.tensor_scalar_mul(
                        v_all[:, at * VD : (at + 1) * VD],
                        v_all[:, at * VD : (at + 1) * VD],
                        1.0 if noscale else f_sb[:, p : p + 1],
                    ).then_inc(s_vsc, 1)

    nc.finalize()
    return nc


# ------------------------- host side -------------------------

def _split_bf16(x):
    import ml_dtypes

    hi = x.astype(ml_dtypes.bfloat16)
    lo = (x - hi.astype(np.float32)).astype(ml_dtypes.bfloat16)
    return hi, lo


def _tile_cols(xT, w):
    """[d, s] -> [128, (d/128)*w] with col = dd*w + i (s == w per d-tile)."""
    dd = xT.shape[0] // 128
    return np.ascontiguousarray(
        xT.reshape(dd, 128, w).transpose(1, 0, 2).reshape(128, dd * w)
    )


def build_in_maps2(inputs, seq=8192, d=1024):
    import ml_dtypes

    bf = ml_dtypes.bfloat16
    SLAB = seq // C
    NDT = d // 128

    qw = (inputs["query_weights"] / np.sqrt(np.float32(QK))).astype(np.float32)
    wqh, wql = _split_bf16(qw)
    wkh, wkl = _split_bf16(inputs["key_weights"].astype(np.float32))

    def dup_tile(w):
        w2 = np.concatenate([w.astype(np.float32)] * 2, axis=1)  # [d, 128]
        return _tile_cols(w2, 128).astype(bf)

    w_maps = {
        "wq2h": dup_tile(wqh), "wq2l": dup_tile(wql),
        "wk2h": dup_tile(wkh), "wk2l": dup_tile(wkl),
        "wv": _tile_cols(
            inputs["value_weights"].astype(np.float32), VD
        ).astype(bf),
    }

    def slab_hi_lo(xT):
        """[d, SLAB] -> [128, NDT*2*SLAB] with col = dd*2*SLAB + h*SLAB + i."""
        hi, lo = _split_bf16(xT)
        ht = _tile_cols(hi.astype(np.float32), SLAB).reshape(128, NDT, SLAB)
        lt = _tile_cols(lo.astype(np.float32), SLAB).reshape(128, NDT, SLAB)
        return np.ascontiguousarray(
            np.concatenate([ht[:, :, None, :], lt[:, :, None, :]], axis=2)
            .reshape(128, NDT * 2 * SLAB)
        ).astype(bf)

    def _vin_tile(xT):
        """[d, SLAB] -> [128, NST*NDT*128], col = st*NDT*128 + dd*128 + jj."""
        t = _tile_cols(xT, SLAB).reshape(128, NDT, SLAB // 128, 128)
        return np.ascontiguousarray(
            t.transpose(0, 2, 1, 3).reshape(128, -1)
        )

    qT = np.ascontiguousarray(inputs["queries"].T).astype(np.float32)
    kT = np.ascontiguousarray(inputs["keys"].T).astype(np.float32)
    vT = np.ascontiguousarray(inputs["values"].T).astype(np.float32)

    in_maps = []
    for c in range(C):
        sl = slice(c * SLAB, (c + 1) * SLAB)
        m = {
            "qin": slab_hi_lo(qT[:, sl]),
            "kin": slab_hi_lo(kT[:, sl]),
            "vin": _vin_tile(vT[:, sl]).astype(bf),
        }
        m.update(w_maps)
        in_maps.append(m)
    return in_maps


def assemble_out2(results, seq=8192):
    SLAB = seq // C
    full = np.zeros((seq, VD), np.float32)
    for c in range(C):
        o = np.asarray(results[c]["out"], dtype=np.float32)  # [VD, SLAB]
        full[c * SLAB : (c + 1) * SLAB] = o.T
    return full


def run_spmd_staged(nc, in_maps, profile_dir=None):
    """run_bass_via_pjrt with inputs pre-staged on-device (blocks until all
    shards are resident) so the 8 cores launch aligned instead of staggered
    by per-device input-transfer time."""
    import jax
    import numpy as np_
    from jax.sharding import Mesh, PartitionSpec, NamedSharding
    from jax.experimental.shard_map import shard_map
    import concourse.mybir as mybir
    from concourse import bass2jax

    bass2jax.install_neuronx_cc_hook()
    n_cores = len(in_maps)

    partition_name = (
        nc.partition_id_tensor.name if nc.partition_id_tensor else None
    )
    in_names, out_names, out_avals, zero_outs = [], [], [], []
    for alloc in nc.m.functions[0].allocations:
        if not isinstance(alloc, mybir.MemoryLocationSet):
            continue
        name = alloc.memorylocations[0].name
        if alloc.kind == "ExternalInput":
            if name != partition_name:
                in_names.append(name)
        elif alloc.kind == "ExternalOutput":
            out_names.append(name)
            shape = tuple(alloc.tensor_shape)
            dtype = mybir.dt.np(alloc.dtype)
            out_avals.append(jax.core.ShapedArray(shape, dtype))
            zero_outs.append(np_.zeros(shape, dtype))
    n_params = len(in_names)
    n_outs = len(out_avals)
    all_names = in_names + out_names
    if partition_name is not None:
        all_names = all_names + [partition_name]

    def _body(*args):
        operands = list(args)
        if partition_name is not None:
            operands.append(bass2jax.partition_id_tensor())
        outs = bass2jax._bass_exec_p.bind(
            *operands,
            out_avals=tuple(out_avals),
            in_names=tuple(all_names),
            out_names=tuple(out_names),
            lowering_input_output_aliases=(),
            sim_require_finite=True,
            sim_require_nnan=True,
            nc=nc,
        )
        return tuple(outs)

    devices = jax.devices()[:n_cores]
    mesh = Mesh(np_.asarray(devices), ("core",))
    spec = NamedSharding(mesh, PartitionSpec("core"))
    sharded = jax.jit(
        shard_map(
            _body,
            mesh=mesh,
            in_specs=(PartitionSpec("core"),) * (n_params + n_outs),
            out_specs=(PartitionSpec("core"),) * n_outs,
            check_rep=False,
        ),
        keep_unused=True,
    )
    concat_in = [
        np_.concatenate([np_.asarray(in_maps[c][nm]) for c in range(n_cores)], axis=0)
        for nm in in_names
    ]
    concat_zero = [
        np_.zeros((n_cores * z.shape[0], *z.shape[1:]), z.dtype) for z in zero_outs
    ]
    staged = [jax.device_put(a, spec) for a in concat_in + concat_zero]
    jax.block_until_ready(staged)

    if profile_dir is not None:
        from antenv.axon_hooks import get_axon_ntff_profile_hook

        hook = get_axon_ntff_profile_hook()
        with hook(profile_dir, list(range(n_cores))):
            out_arrs = sharded(*staged)
            jax.block_until_ready(out_arrs)
    else:
        out_arrs = sharded(*staged)
    return [
        {
            nm: np_.asarray(out_arrs[i]).reshape(n_cores, *out_avals[i].shape)[c]
            for i, nm in enumerate(out_names)
        }
        for c in range(n_cores)
    ]


def kernel(queries, keys, values, query_weights, key_weights, value_weights):
    import sys

    for p in ("/opt/trn_rl_repo",):
        if p not in sys.path:
            sys.path.insert(0, p)

    seq, d = queries.shape
    inputs = {
        "queries": queries, "keys": keys, "values": values,
        "query_weights": query_weights, "key_weights": key_weights,
        "value_weights": value_weights,
    }
    in_maps = build_in_maps2(inputs, seq=seq, d=d)
    nc = build_nc2(seq=seq, d=d)
    results = run_spmd_staged(nc, in_maps)
    return assemble_out2(results, seq=seq)


# revision 35
# speedup vs baseline: 13.2047x; 13.2047x over previous
"""Sequence-parallel Trainium2 attention-head kernel (softmax over queries).

Shard the QUERY dim across the 8 cores (slab = 1024 queries each); every
core computes scores for ALL 8192 keys x its own query slab with j (keys)
on partitions, so the softmax-over-queries stats are per-partition-row:

  out[i,:] = sum_j exp(s_ij - M_j)/D_j * v[j,:],  M_j/D_j global over i.

Per core:
  1. Project its k/v slab (j on partitions) -> AllGather #1 (384 KB/core)
     so every core holds full projected KS=[kh;kl] and v.  Project its q
     slab -> Q1=[qh;ql], Q2=[ql;qh] (weights host-duplicated into both
     column halves so the fp32 projection lands on all 128 partitions in
     one pass; hi/lo split is then partition-aligned DVE).
  2. Per j-tile t (128 keys x 1024 local queries): 2 stacked matmuls
     (KS_t^T Q1 + KS_t^T Q2 = all 4 hi/lo cross terms), DVE negated
     row-max -> bias, ACT exp (psum->bf16) with accum_out -> local sums.
     Bias is the LOCAL max over this core's slab, so the per-row rescale
     exp(b_local - M)/D folds entirely into v later.
  3. AllGather #2 of the per-(j,core) stats [negmax|sum] (64 KB/core);
     every core reduces them to global M_j, D_j, rescales v rows by
     f_j = exp(b_local_j - M_j)/D_j, then accumulates the 64 attn
     matmuls v'_t^T e_t into psum [64 v, 1024 i] and writes its slab.
"""

import numpy as np

C = 8
QK = 64
VD = 64


def build_nc2(seq=8192, d=1024, reps=1, warmup=True, dbg=False, noscale=False, attn_lite=False, scores_lite=False, no_cc=False, exp_lite=False, max_lite=False):
    import concourse.bacc as bacc
    import concourse.mybir as mybir

    f32 = mybir.dt.float32
    bf16 = mybir.dt.bfloat16
    AX = mybir.AxisListType.X
    ALU = mybir.AluOpType
    ACTF = mybir.ActivationFunctionType

    NDT = d // 128            # 8 d-tiles
    SLAB = seq // C           # 1024 queries per core
    NT = seq // 128           # 64 global j-tiles
    NST = SLAB // 128         # 8 v-proj subtiles per slab
    SS = max(SLAB, 512)       # psum slot stride (bank-aligned)
    NDMA = 3 * NDT            # arena chunk DMAs per iteration (k, v, q)
    INF = 3.0e38

    nc = bacc.Bacc(target_bir_lowering=False, debug=False, num_devices=C)

    def din(name, w, dt=bf16):
        return nc.declare_dram_parameter(name, [128, w], dt, isOutput=False)

    qin_d = din("qin", NDT * 2 * SLAB)     # col = dd*2048 + h*1024 + i
    kin_d = din("kin", NDT * 2 * SLAB)
    vin_d = din("vin", NDT * SLAB)         # col = dd*1024 + j
    wq2h_d, wq2l_d = din("wq2h", NDT * 128), din("wq2l", NDT * 128)
    wk2h_d, wk2l_d = din("wk2h", NDT * 128), din("wk2l", NDT * 128)
    wv_d = din("wv", NDT * VD)
    out_d = nc.declare_dram_parameter("out", [VD, SLAB], f32, isOutput=True)
    if dbg:
        dKS_d = nc.declare_dram_parameter("dKS", [128, seq], bf16, isOutput=True)
        dQ1_d = nc.declare_dram_parameter("dQ1", [128, SLAB], bf16, isOutput=True)
        dQ2_d = nc.declare_dram_parameter("dQ2", [128, SLAB], bf16, isOutput=True)
        dst_d = nc.declare_dram_parameter("dst", [128, 2 * NT], f32, isOutput=True)
        dsa_d = nc.declare_dram_parameter("dsa", [128, C * 2 * NT], f32, isOutput=True)
        df_d = nc.declare_dram_parameter("df", [128, NT], f32, isOutput=True)
        dvl_d = nc.declare_dram_parameter("dvl", [128, SLAB // 2], bf16, isOutput=True)
        dva_d = nc.declare_dram_parameter("dva", [128, NT * VD], bf16, isOutput=True)
        de_d = nc.declare_dram_parameter("de", [128, 4 * SLAB], bf16, isOutput=True)

    # collective bounce buffers (DRAM; collectives can't touch I/O tensors)
    NKG = min(4, SLAB // 128) # KS gather chunks (pipelined collectives)
    KGW = SLAB // NKG         # j-columns per chunk
    cc1_ins = [
        nc.dram_tensor(f"cc1i_{g}", [128, KGW], bf16) for g in range(NKG)
    ]
    ccv_in = nc.dram_tensor("ccv_in", [128, SLAB // 2], bf16)
    cc1_outs = [
        nc.dram_tensor(f"cc1o_{g}", [C * 128, KGW], bf16, addr_space="Shared")
        for g in range(NKG)
    ]
    ccv_out = nc.dram_tensor(
        "ccv_out", [C * 128, SLAB // 2], bf16, addr_space="Shared"
    )
    cc2_ins = [nc.dram_tensor(f"cc2i_{h}", [128, NT], f32) for h in range(2)]
    cc2_outs2 = [
        nc.dram_tensor(f"cc2o_{h}", [C * 128, NT], f32, addr_space="Shared")
        for h in range(2)
    ]
    HW2 = NT // 2

    from contextlib import ExitStack

    with ExitStack() as ctx:
        block = ctx.enter_context(nc.Block())
        sem = lambda n: ctx.enter_context(nc.semaphore(n))
        sb = lambda n, shape, dt: ctx.enter_context(nc.sbuf_tensor(n, shape, dt))
        ps = lambda n, shape: ctx.enter_context(nc.psum_tensor(n, shape, f32))

        s_w = sem("s_w")        # weight DMAs: 80 once
        s_ka = [sem(f"s_ka{i}") for i in range(NDT)]   # per-chunk k DMA
        s_va = [sem(f"s_va{i}") for i in range(NST)]   # per-chunk v DMA
        s_qa = [sem(f"s_qa{i}") for i in range(NDT)]   # per-chunk q DMA
        s_kp = sem("s_kp")      # 8/iter (k proj per dd)
        s_vp = sem("s_vp")      # 8/iter
        s_qp = sem("s_qp")      # 8/iter
        s_ks = sem("s_ks")      # 1/iter k split done
        s_vsp = sem("s_vsp")    # 1/iter vloc copy done
        s_qs = sem("s_qs")      # 1/iter q split done
        s_gb = sem("s_gb")      # 32/iter bounce-in DMAs
        s_cc1 = sem("s_cc1")    # 1/iter
        s_ccd = sem("s_ccd")    # 256/iter gather-back DMAs
        s_sc = sem("s_sc")      # 64/iter score tiles
        s_mx = sem("s_mx")      # 64/iter
        s_ex = sem("s_ex")      # 64/iter
        s_g2 = sem("s_g2")      # 16/iter stats bounce DMA
        s_cc2 = sem("s_cc2")    # 1/iter
        s_std = sem("s_std")    # 128/iter stats gather-back
        s_sm = sem("s_sm")      # 1/iter DVE->ACT stats handoff
        s_sa = sem("s_sa")      # 2/iter ACT->DVE stats handoff
        s_f = sem("s_f")        # 2/iter DVE f ready (per half)
        s_vsc = sem("s_vsc")    # 64/iter v rescales
        s_at = sem("s_at")      # 1/iter attn done
        s_oc = sem("s_oc")      # 1/iter out copy done
        s_out = sem("s_out")    # 16/iter out DMA
        s_ch = sem("s_ch")      # DVE same-engine RAW chain

        arena_k = sb("arena_k", [128, 2 * 2 * SLAB], bf16)  # dd%2 ping-pong
        arena_v = sb("arena_v", [128, 2 * NDT * 128], bf16)
        arena_q = sb("arena_q", [128, 2 * 2 * SLAB], bf16)
        wq2h = sb("wq2h_s", [128, NDT * 128], bf16)
        wq2l = sb("wq2l_s", [128, NDT * 128], bf16)
        wk2h = sb("wk2h_s", [128, NDT * 128], bf16)
        wk2l = sb("wk2l_s", [128, NDT * 128], bf16)
        wv = sb("wv_s", [128, NDT * VD], bf16)
        Q1 = sb("Q1", [128, SLAB], bf16)
        Q2 = sb("Q2", [128, SLAB], bf16)
        KSloc = sb("KSloc", [128, SLAB], bf16)
        ktmp = sb("ktmp", [128, SLAB], bf16)
        vloc = sb("vloc", [128, SLAB // 2], bf16)
        KS = sb("KS", [128, seq], bf16)
        v_all = sb("v_all", [128, NT * VD], bf16)
        e_sb = sb("e_sb", [128, NT * SLAB], bf16)           # 128 KB/part
        stats_loc = sb("stats_loc", [128, 2 * NT], f32)     # [negmax | sum]
        stats_all = sb("stats_all", [128, C * 2 * NT], f32)
        NM = sb("NM", [128, NT], f32)
        darg = sb("darg", [128, C * NT], f32)
        w8 = sb("w8", [128, C * NT], f32)
        wD = sb("wD", [128, C * NT], f32)
        Dt = sb("Dt", [128, NT], f32)
        Rt = sb("Rt", [128, NT], f32)
        wlarg = sb("wlarg", [128, NT], f32)
        wl = sb("wl", [128, NT], f32)
        f_sb = sb("f_sb", [128, NT], f32)

        out_sb = sb("out_sb", [VD, SLAB], f32)
        ps_s = ps("ps_s", [128, 3 * SS])     # proj + score slots (6 banks)
        ps_o = ps("ps_o", [VD, SLAB])        # 2 banks
        ps_v = ps_s[:, 2 * SS : 2 * SS + SLAB // 2]   # v proj in slot 2

        TORD = [
            c * (SLAB // 128) + gi * (KGW // 128) + u
            for gi in range(NKG)
            for c in range(C)
            for u in range(KGW // 128)
        ]

        # ---------------- SYNC: k/v input + output DMAs ----------------
        # per-chunk s_in levels must be ordered sync points, so each arena
        # DMA is serialized (waited) before the next is issued.
        @block.sync
        def _(s):
          for it in range(reps):
            for dd in range(NDT):          # k chunks
                g = it * NDT + dd
                if g >= 2:
                    s.wait_ge(s_kp, g - 1)
                s.dma_start(
                    out=arena_k[:, (dd % 2) * 2 * SLAB : (dd % 2 + 1) * 2 * SLAB],
                    in_=kin_d[:, dd * 2 * SLAB : (dd + 1) * 2 * SLAB],
                ).then_inc(s_ka[dd], 16)
            for st in range(NST):          # v chunks (one j-subtile, all dd)
                g = it * NST + st
                if g >= 2:
                    s.wait_ge(s_vp, g - 1)
                s.dma_start(
                    out=arena_v[:, (st % 2) * NDT * 128 : (st % 2 + 1) * NDT * 128],
                    in_=vin_d[:, st * NDT * 128 : (st + 1) * NDT * 128],
                ).then_inc(s_va[st], 16)
            s.wait_ge(s_oc, it + 1)
            s.dma_start(out=out_d[:, :], in_=out_sb[:, :]).then_inc(s_out, 16)
            s.wait_ge(s_out, it * 16 + 16)
            if dbg and it == reps - 1:
                n_d = 0
                for dst, srcb in ((dKS_d, KS), (dQ1_d, Q1), (dQ2_d, Q2),
                                  (dst_d, stats_loc), (dsa_d, stats_all),
                                  (df_d, f_sb), (dva_d, v_all),
                                  (dvl_d, vloc)):
                    s.dma_start(out=dst[:, :], in_=srcb[:, :]).then_inc(s_out, 16)
                    n_d += 1
                s.dma_start(
                    out=de_d[:, :], in_=e_sb[:, 0 : 4 * SLAB]
                ).then_inc(s_out, 16)
                n_d += 1
                s.wait_ge(s_out, it * 16 + 16 + n_d * 16)

        # ---------------- TENSOR (PE) ----------------
        @block.tensor
        def _(t):
          for it in range(reps):
            for w_ in range(40 if (warmup and it == 0) else 0):
                t.matmul(
                    ps_s[0:64, 0:512], Q1[:, 0:64], Q1[:, 0 : min(SLAB, 512)],
                    start=(w_ == 0), stop=False,
                )
            if it == 0:
                t.wait_ge(s_w, 80)
            # ---- k proj -> ps_s[:, 0:SLAB] (both halves via dup weights)
            if it > 0:
                t.wait_ge(s_ex, it * NT)       # all prev-rep slots free
            for dd in range(NDT):
                t.wait_ge(s_ka[dd], (it + 1) * 16)
                kb = (dd % 2) * 2 * SLAB
                uh = arena_k[:, kb : kb + SLAB]
                ul = arena_k[:, kb + SLAB : kb + 2 * SLAB]
                for ti, (W, X) in enumerate(((wk2h, uh), (wk2h, ul), (wk2l, uh))):
                    for hb in range(0, SLAB, 512):
                        he = min(hb + 512, SLAB)
                        mm = t.matmul(
                            ps_s[:, hb:he],
                            W[:, dd * 128 : (dd + 1) * 128], X[:, hb:he],
                            start=(dd == 0 and ti == 0),
                            stop=(dd == NDT - 1 and ti == 2),
                            skip_group_check=True,
                        )
                mm.then_inc(s_kp, 1)
            # ---- v proj -> ps_v (st-outer: one sequential group per
            # subtile; vin is re-tiled so chunk st holds all dd for its j's)
            if it > 0:
                t.wait_ge(s_vsp, it)           # ps_v free
            for st in range(NST):
                t.wait_ge(s_va[st], (it + 1) * 16)
                vb = (st % 2) * NDT * 128
                for dd in range(NDT):
                    mm = t.matmul(
                        ps_v[:, st * VD : (st + 1) * VD],
                        arena_v[:, vb + dd * 128 : vb + (dd + 1) * 128],
                        wv[:, dd * VD : (dd + 1) * VD],
                        start=(dd == 0), stop=(dd == NDT - 1),
                    )
                mm.then_inc(s_vp, 1)
            # ---- q proj -> ps_s[:, SLAB:2*SLAB]
            if it > 0:
                t.wait_ge(s_ex, it * NT)       # slot1 free (prev tile 63)
            for dd in range(NDT):
                t.wait_ge(s_qa[dd], (it + 1) * 16)
                qb = (dd % 2) * 2 * SLAB
                uh = arena_q[:, qb : qb + SLAB]
                ul = arena_q[:, qb + SLAB : qb + 2 * SLAB]
                for ti, (W, X) in enumerate(((wq2h, uh), (wq2h, ul), (wq2l, uh))):
                    for hb in range(0, SLAB, 512):
                        he = min(hb + 512, SLAB)
                        mm = t.matmul(
                            ps_s[:, SS + hb : SS + he],
                            W[:, dd * 128 : (dd + 1) * 128], X[:, hb:he],
                            start=(dd == 0 and ti == 0),
                            stop=(dd == NDT - 1 and ti == 2),
                            skip_group_check=True,
                        )
                mm.then_inc(s_qp, 1)
            # ---- scores: 64 j-tiles, ordered by gather chunk so tiles of
            # chunk gi run as soon as collective gi has landed
            t.wait_ge(s_qs, it + 1)
            for pos in range(NT):
                tt = TORD[pos]
                gi = pos // (NT // NKG)
                if pos % (NT // NKG) == 0:
                    t.wait_ge(s_ccd, it * (NKG + 1) * 16 + (gi + 1) * 16)
                g = it * NT + pos
                if g >= 3:
                    t.wait_ge(s_ex, g - 2)     # slot free (exp 3 positions ago)
                so = (pos % 3) * SS
                kt = KS[:, tt * 128 : (tt + 1) * 128]
                for hb in range(0, SLAB, 512):
                    he = min(hb + 512, SLAB)
                    if not scores_lite:
                        t.matmul(
                            ps_s[:, so + hb : so + he], kt, Q1[:, hb:he],
                            start=True, stop=False, skip_group_check=True,
                        )
                    mm = t.matmul(
                        ps_s[:, so + hb : so + he], kt, Q2[:, hb:he],
                        start=scores_lite, stop=True, skip_group_check=True,
                    )
                mm.then_inc(s_sc, 1)
                # interleaved attn for the first half's tiles once their
                # stats have come back (pos 58..63 -> attn 0..5)
                if NT >= 32 and pos >= NT - 6:
                    p = pos - (NT - 6)
                    at = TORD[p]
                    if p == 0:
                        t.wait_ge(s_oc, it)    # ps_o free
                    t.wait_ge(s_vsc, it * NT + p + 1)
                    for hb in range(0, SLAB, 512):
                        he = min(hb + 512, SLAB)
                        t.matmul(
                            ps_o[:, hb:he],
                            v_all[:, at * VD : (at + 1) * VD],
                            e_sb[:, at * SLAB + hb : at * SLAB + he],
                            start=(p == 0), stop=False,
                            skip_group_check=True,
                        )
            # ---- attn tail: remaining tiles
            for p in range(6 if NT >= 32 else 0, NT):
                at = TORD[p]
                t.wait_ge(s_vsc, it * NT + p + 1)
                if p == 0:
                    t.wait_ge(s_oc, it)        # ps_o free
                for hb in range(0, SLAB, 512):
                    he = min(hb + 512, SLAB)
                    mm = t.matmul(
                        ps_o[:, hb:he],
                        v_all[:, at * VD : (at + 1) * VD],
                        e_sb[:, at * SLAB + hb : at * SLAB + he],
                        start=(p == 0), stop=(p == NT - 1),
                        skip_group_check=True,
                    )
            mm.then_inc(s_at, 1)

        # ---------------- VECTOR (DVE) ----------------
        ch_n = [0]

        @block.vector
        def _(v):
          def step(inst):
              inst.then_inc(s_ch, 1)
              ch_n[0] += 1

          def cw(v):
              v.wait_ge(s_ch, ch_n[0])

          for it in range(reps):
            # k split -> KSloc = [kh; kl]
            v.wait_ge(s_kp, it * NDT + NDT)
            if it > 0:
                v.wait_ge(s_gb, it * (NKG + 1) * 16)   # KSloc/vloc consumed
            step(v.tensor_copy(KSloc[0:64, :], ps_s[0:64, 0:SLAB]))
            step(v.tensor_copy(ktmp[64:128, :], ps_s[64:128, 0:SLAB]))
            cw(v)
            v.tensor_tensor(
                KSloc[64:128, :], ps_s[64:128, 0:SLAB], ktmp[64:128, :],
                op=ALU.subtract,
            ).then_inc(s_ks, 1)
            # v copy
            v.wait_ge(s_vp, it * NST + NST)
            v.tensor_copy(vloc[:, :], ps_v[:, :]).then_inc(s_vsp, 1)
            # q split -> Q1=[qh;ql], Q2=[ql;qh]
            v.wait_ge(s_qp, it * NDT + NDT)
            qsl = ps_s[:, SS : SS + SLAB]
            step(v.tensor_copy(Q1[0:64, :], qsl[0:64, :]))
            step(v.tensor_copy(Q2[64:128, :], qsl[64:128, :]))
            cw(v)
            v.tensor_tensor(
                Q1[64:128, :], qsl[64:128, :], Q2[64:128, :], op=ALU.subtract
            )
            v.tensor_tensor(
                Q2[0:64, :], qsl[0:64, :], Q1[0:64, :], op=ALU.subtract
            ).then_inc(s_qs, 1)
            # per-tile negated row-max: one fused pass over both psum
            # halves: scr = -max(lo, hi) elementwise, nb = min(scr)
            def part1(h):
                v.wait_ge(s_std, it * 64 + h * 32 + 32)
                nb_v = stats_all[:, :].rearrange(
                    "p (c t) -> p t c", c=C, t=2 * NT
                )
                hs = slice(h * HW2, (h + 1) * HW2)
                step(v.tensor_reduce(
                    NM[:, hs], nb_v[:, h * HW2 : (h + 1) * HW2, :],
                    axis=AX, op=ALU.min,
                ))
                cw(v)
                for c in range(C):
                    v.tensor_tensor(
                        darg[:, c * NT + h * HW2 : c * NT + (h + 1) * HW2],
                        NM[:, hs],
                        stats_all[:, c * 2 * NT + h * HW2 : c * 2 * NT + (h + 1) * HW2],
                        op=ALU.subtract,
                    )
                v.tensor_tensor(
                    wlarg[:, hs], NM[:, hs], stats_loc[:, hs], op=ALU.subtract
                ).then_inc(s_sm, 1)

            def part2(h):
                v.wait_ge(s_sa, it * 2 + h + 1)
                hs = slice(h * HW2, (h + 1) * HW2)
                dl_v = stats_all[:, :].rearrange(
                    "p (c t) -> p c t", c=C, t=2 * NT
                )
                step(v.tensor_tensor(
                    wD[:, :].rearrange("p (c t) -> p c t", c=C, t=NT)[
                        :, :, h * HW2 : (h + 1) * HW2
                    ],
                    w8[:, :].rearrange("p (c t) -> p c t", c=C, t=NT)[
                        :, :, h * HW2 : (h + 1) * HW2
                    ],
                    dl_v[:, :, NT + h * HW2 : NT + (h + 1) * HW2],
                    op=ALU.mult,
                ))
                cw(v)
                step(v.tensor_reduce(
                    Dt[:, hs],
                    wD[:, :].rearrange("p (c t) -> p t c", c=C, t=NT)[
                        :, h * HW2 : (h + 1) * HW2, :
                    ],
                    axis=AX, op=ALU.add,
                ))
                cw(v)
                step(v.reciprocal(Rt[:, hs], Dt[:, hs]))
                cw(v)
                step(v.tensor_tensor(
                    f_sb[:, hs], wl[:, hs], Rt[:, hs], op=ALU.mult
                ))
                cw(v)
                v.nop().then_inc(s_f, 1)

            for pos in range(NT):
                v.wait_ge(s_sc, it * NT + pos + 1)
                if pos == 0 and it > 0:
                    v.wait_ge(s_g2, it * 64)   # stats_loc consumed
                    v.wait_ge(s_at, it)        # e_sb junk-write target free
                so = (pos % 3) * SS
                v.tensor_reduce(
                    stats_loc[:, pos : pos + 1],
                    ps_s[:, so : so + (SLAB // 2 if max_lite else SLAB)],
                    axis=AX, op=ALU.max, negate=True,
                ).then_inc(s_mx, 1)
                if NT >= 64 and pos == 52:
                    part1(0)
                if NT >= 64 and pos == 56:
                    part2(0)
            if NT < 64:
                part1(0)
                part2(0)
            part1(1)
            part2(1)

        # ---------------- SCALAR (ACT) ----------------
        @block.scalar
        def _(sc):
          for it in range(reps):
            # q arena stream on the ACT queue (serialized per-chunk levels)
            for dd in range(NDT):
                g = it * NDT + dd
                if g >= 2:
                    sc.wait_ge(s_qp, g - 1)
                sc.dma_start(
                    out=arena_q[:, (dd % 2) * 2 * SLAB : (dd % 2 + 1) * 2 * SLAB],
                    in_=qin_d[:, dd * 2 * SLAB : (dd + 1) * 2 * SLAB],
                ).then_inc(s_qa[dd], 16)
            def half_exps(h):
                sc.wait_ge(s_sm, it * 2 + h + 1)
                hs = slice(h * HW2, (h + 1) * HW2)
                sc.activation(
                    w8[:, :].rearrange("p (c t) -> p c t", c=C, t=NT)[
                        :, :, h * HW2 : (h + 1) * HW2
                    ],
                    darg[:, :].rearrange("p (c t) -> p c t", c=C, t=NT)[
                        :, :, h * HW2 : (h + 1) * HW2
                    ],
                    ACTF.Exp,
                )
                sc.activation(wl[:, hs], wlarg[:, hs], ACTF.Exp).then_inc(
                    s_sa, 1
                )

            for pos in range(NT):
                tt = TORD[pos]
                sc.wait_ge(s_mx, it * NT + pos + 1)
                if pos == 0 and it > 0:
                    sc.wait_ge(s_at, it)       # e_sb consumed by attn
                ew = SLAB // 2 if exp_lite else SLAB
                sc.activation(
                    e_sb[:, tt * SLAB : tt * SLAB + ew],
                    ps_s[:, (pos % 3) * SS : (pos % 3) * SS + ew],
                    ACTF.Exp,
                    bias=stats_loc[:, pos : pos + 1], scale=1.0,
                    accum_out=stats_loc[:, NT + pos : NT + pos + 1],
                ).then_inc(s_ex, 1)
                if NT >= 64 and pos == 54:
                    half_exps(0)
            if NT < 64:
                half_exps(0)
            half_exps(1)
            sc.wait_ge(s_at, it + 1)
            if it > 0:
                sc.wait_ge(s_out, it * 16)     # out_sb consumed
            sc.activation(out_sb[:, :], ps_o[:, :], ACTF.Copy).then_inc(s_oc, 1)

        # ---------------- GPSIMD: collectives ----------------
        @block.gpsimd
        def _(g):
          for it in range(reps):
            if it == 0:
                for wsb, wdr in ((wq2h, wq2h_d), (wq2l, wq2l_d),
                                 (wk2h, wk2h_d), (wk2l, wk2l_d), (wv, wv_d)):
                    g.dma_start(out=wsb[:, :], in_=wdr[:, :]).then_inc(s_w, 16)
            g.wait_ge(s_ks, it + 1)
            if it > 0:
                g.wait_ge(s_cc1, it * (NKG + 1))   # cc1_in consumed
            for gi in range(NKG):
                g.dma_start(
                    out=cc1_ins[gi][:, :],
                    in_=KSloc[:, gi * KGW : (gi + 1) * KGW],
                ).then_inc(s_gb, 16)
            g.wait_ge(s_gb, it * (NKG + 1) * 16 + NKG * 16)
            for gi in range(NKG):
                if no_cc:
                    g.nop().then_inc(s_cc1, 1)
                else:
                    g.collective_compute(
                        "AllGather", mybir.AluOpType.bypass,
                        replica_groups=[list(range(C))],
                        ins=[cc1_ins[gi][:, :].opt()],
                        outs=[cc1_outs[gi][:, :].opt()],
                    ).then_inc(s_cc1, 1)
                g.wait_ge(s_cc1, it * (NKG + 1) + gi + 1)
                g.wait_ge(s_ccd, it * (NKG + 1) * 16 + gi * 16)
                g.dma_start(
                    out=KS[:, :].rearrange("p (c s) -> p c s", c=C)[
                        :, :, gi * KGW : (gi + 1) * KGW
                    ],
                    in_=cc1_outs[gi][:, :].rearrange("(c p) w -> p c w", c=C),
                ).then_inc(s_ccd, 16)
            g.wait_ge(s_vsp, it + 1)
            g.dma_start(out=ccv_in[:, :], in_=vloc[:, :]).then_inc(s_gb, 16)
            g.wait_ge(s_gb, (it + 1) * (NKG + 1) * 16)
            if no_cc:
                g.nop().then_inc(s_cc1, 1)
            else:
                g.collective_compute(
                    "AllGather", mybir.AluOpType.bypass,
                    replica_groups=[list(range(C))],
                    ins=[ccv_in[:, :].opt()],
                    outs=[ccv_out[:, :].opt()],
                ).then_inc(s_cc1, 1)
            g.wait_ge(s_cc1, it * (NKG + 1) + NKG + 1)
            g.wait_ge(s_ccd, it * (NKG + 1) * 16 + NKG * 16)
            g.dma_start(
                out=v_all[:, :].rearrange("p (c s) -> p c s", c=C),
                in_=ccv_out[:, :].rearrange("(c p) w -> p c w", c=C),
            ).then_inc(s_ccd, 16)
            # stats gather + v rescale, per half (overlaps the score phase)
            sa_v = stats_all[:, :].rearrange("p (c s) -> p c s", c=C)
            for h in range(2):
                g.wait_ge(s_ex, it * NT + (h + 1) * HW2)
                if it > 0:
                    g.wait_ge(s_cc2, it * 2)   # cc2_ins free
                g.dma_start(
                    out=cc2_ins[h][:, 0:HW2],
                    in_=stats_loc[:, h * HW2 : (h + 1) * HW2],
                ).then_inc(s_g2, 16)
                g.dma_start(
                    out=cc2_ins[h][:, HW2 : 2 * HW2],
                    in_=stats_loc[:, NT + h * HW2 : NT + (h + 1) * HW2],
                ).then_inc(s_g2, 16)
                g.wait_ge(s_g2, it * 64 + h * 32 + 32)
                if no_cc:
                    g.nop().then_inc(s_cc2, 1)
                else:
                    g.collective_compute(
                        "AllGather", mybir.AluOpType.bypass,
                        replica_groups=[list(range(C))],
                        ins=[cc2_ins[h][:, :].opt()],
                        outs=[cc2_outs2[h][:, :].opt()],
                    ).then_inc(s_cc2, 1)
                g.wait_ge(s_cc2, it * 2 + h + 1)
                g.wait_ge(s_std, it * 64 + h * 32)
                co_v = cc2_outs2[h][:, :].rearrange("(c p) w -> p c w", c=C)
                g.dma_start(
                    out=sa_v[:, :, h * HW2 : (h + 1) * HW2],
                    in_=co_v[:, :, 0:HW2],
                ).then_inc(s_std, 16)
                g.dma_start(
                    out=sa_v[:, :, NT + h * HW2 : NT + (h + 1) * HW2],
                    in_=co_v[:, :, HW2 : 2 * HW2],
                ).then_inc(s_std, 16)
                # v rescale for this half's tiles
                g.wait_ge(s_f, it * 2 + h + 1)
                if h == 0:
                    g.wait_ge(s_ccd, (it + 1) * (NKG + 1) * 16)  # v_all landed
                for p in range(h * HW2, (h + 1) * HW2):
                    at = TORD[p]
                    g.tensor_scalar_mul(
                        v_all[:, at * VD : (at + 1) * VD],
                        v_all[:, at * VD : (at + 1) * VD],
                        1.0 if noscale else f_sb[:, p : p + 1],
                    ).then_inc(s_vsc, 1)

    nc.finalize()
    return nc


# ------------------------- host side -------------------------

def _split_bf16(x):
    import ml_dtypes

    hi = x.astype(ml_dtypes.bfloat16)
    lo = (x - hi.astype(np.float32)).astype(ml_dtypes.bfloat16)
    return hi, lo


def _tile_cols(xT, w):
    """[d, s] -> [128, (d/128)*w] with col = dd*w + i (s == w per d-tile)."""
    dd = xT.shape[0] // 128
    return np.ascontiguousarray(
        xT.reshape(dd, 128, w).transpose(1, 0, 2).reshape(128, dd * w)
    )


def build_in_maps2(inputs, seq=8192, d=1024):
    import ml_dtypes

    bf = ml_dtypes.bfloat16
    SLAB = seq // C
    NDT = d // 128

    qw = (inputs["query_weights"] / np.sqrt(np.float32(QK))).astype(np.float32)
    wqh, wql = _split_bf16(qw)
    wkh, wkl = _split_bf16(inputs["key_weights"].astype(np.float32))

    def dup_tile(w):
        w2 = np.concatenate([w.astype(np.float32)] * 2, axis=1)  # [d, 128]
        return _tile_cols(w2, 128).astype(bf)

    w_maps = {
        "wq2h": dup_tile(wqh), "wq2l": dup_tile(wql),
        "wk2h": dup_tile(wkh), "wk2l": dup_tile(wkl),
        "wv": _tile_cols(
            inputs["value_weights"].astype(np.float32), VD
        ).astype(bf),
    }

    def slab_hi_lo(xT):
        """[d, SLAB] -> [128, NDT*2*SLAB] with col = dd*2*SLAB + h*SLAB + i."""
        hi, lo = _split_bf16(xT)
        ht = _tile_cols(hi.astype(np.float32), SLAB).reshape(128, NDT, SLAB)
        lt = _tile_cols(lo.astype(np.float32), SLAB).reshape(128, NDT, SLAB)
        return np.ascontiguousarray(
            np.concatenate([ht[:, :, None, :], lt[:, :, None, :]], axis=2)
            .reshape(128, NDT * 2 * SLAB)
        ).astype(bf)

    def _vin_tile(xT):
        """[d, SLAB] -> [128, NST*NDT*128], col = st*NDT*128 + dd*128 + jj."""
        t = _tile_cols(xT, SLAB).reshape(128, NDT, SLAB // 128, 128)
        return np.ascontiguousarray(
            t.transpose(0, 2, 1, 3).reshape(128, -1)
        )

    qT = np.ascontiguousarray(inputs["queries"].T).astype(np.float32)
    kT = np.ascontiguousarray(inputs["keys"].T).astype(np.float32)
    vT = np.ascontiguousarray(inputs["values"].T).astype(np.float32)

    in_maps = []
    for c in range(C):
        sl = slice(c * SLAB, (c + 1) * SLAB)
        m = {
            "qin": slab_hi_lo(qT[:, sl]),
            "kin": slab_hi_lo(kT[:, sl]),
            "vin": _vin_tile(vT[:, sl]).astype(bf),
        }
        m.update(w_maps)
        in_maps.append(m)
    return in_maps


def assemble_out2(results, seq=8192):
    SLAB = seq // C
    full = np.zeros((seq, VD), np.float32)
    for c in range(C):
        o = np.asarray(results[c]["out"], dtype=np.float32)  # [VD, SLAB]
        full[c * SLAB : (c + 1) * SLAB] = o.T
    return full


def run_spmd_staged(nc, in_maps, profile_dir=None):
    """run_bass_via_pjrt with inputs pre-staged on-device (blocks until all
    shards are resident) so the 8 cores launch aligned instead of staggered
    by per-device input-transfer time."""
    import jax
    import numpy as np_
    from jax.sharding import Mesh, PartitionSpec, NamedSharding
    from jax.experimental.shard_map import shard_map
    import concourse.mybir as mybir
    from concourse import bass2jax

    bass2jax.install_neuronx_cc_hook()
    n_cores = len(in_maps)

    partition_name = (
        nc.partition_id_tensor.name if nc.partition_id_tensor else None
    )
    in_names, out_names, out_avals, zero_outs = [], [], [], []
    for alloc in nc.m.functions[0].allocations:
        if not isinstance(alloc, mybir.MemoryLocationSet):
            continue
        name = alloc.memorylocations[0].name
        if alloc.kind == "ExternalInput":
            if name != partition_name:
                in_names.append(name)
        elif alloc.kind == "ExternalOutput":
            out_names.append(name)
            shape = tuple(alloc.tensor_shape)
            dtype = mybir.dt.np(alloc.dtype)
            out_avals.append(jax.core.ShapedArray(shape, dtype))
            zero_outs.append(np_.zeros(shape, dtype))
    n_params = len(in_names)
    n_outs = len(out_avals)
    all_names = in_names + out_names
    if partition_name is not None:
        all_names = all_names + [partition_name]

    def _body(*args):
        operands = list(args)
        if partition_name is not None:
            operands.append(bass2jax.partition_id_tensor())
        outs = bass2jax._bass_exec_p.bind(
            *operands,
            out_avals=tuple(out_avals),
            in_names=tuple(all_names),
            out_names=tuple(out_names),
            lowering_input_output_aliases=(),
            sim_require_finite=True,
            sim_require_nnan=True,
            nc=nc,
        )
        return tuple(outs)

    devices = jax.devices()[:n_cores]
    mesh = Mesh(np_.asarray(devices), ("core",))
    spec = NamedSharding(mesh, PartitionSpec("core"))
    sharded = jax.jit(
        shard_map(
            _body,
            mesh=mesh,
            in_specs=(PartitionSpec("core"),) * (n_params + n_outs),
            out_specs=(PartitionSpec("core"),) * n_outs,
            check_rep=False,
        ),
        keep_unused=True,
    )
    concat_in = [
        np_.concatenate([np_.asarray(in_maps[c][nm]) for c in range(n_cores)], axis=0)
        for nm in in_names
    ]
    concat_zero = [
        np_.zeros((n_cores * z.shape[0], *z.shape[1:]), z.dtype) for z in zero_outs
    ]
    staged = [jax.device_put(a, spec) for a in concat_in + concat_zero]
    jax.block_until_ready(staged)

    if profile_dir is not None:
        from antenv.axon_hooks import get_axon_ntff_profile_hook

        hook = get_axon_ntff_profile_hook()
        with hook(profile_dir, list(range(n_cores))):
            out_arrs = sharded(*staged)
            jax.block_until_ready(out_arrs)
    else:
        out_arrs = sharded(*staged)
    return [
        {
            nm: np_.asarray(out_arrs[i]).reshape(n_cores, *out_avals[i].shape)[c]
            for i, nm in enumerate(out_names)
        }
        for c in range(n_cores)
    ]


def kernel(queries, keys, values, query_weights, key_weights, value_weights):
    import sys

    for p in ("/opt/trn_rl_repo",):
        if p not in sys.path:
            sys.path.insert(0, p)

    seq, d = queries.shape
    inputs = {
        "queries": queries, "keys": keys, "values": values,
        "query_weights": query_weights, "key_weights": key_weights,
        "value_weights": value_weights,
    }
    in_maps = build_in_maps2(inputs, seq=seq, d=d)
    nc = build_nc2(seq=seq, d=d)
    results = run_spmd_staged(nc, in_maps)
    return assemble_out2(results, seq=seq)
